# revision 15
# baseline (speedup 1.0000x reference)
"""Trainium2 Bass kernel for nn_Decoder (teacher-forced LSTM decoder).

Contract: kernel(**inputs) takes the FULL unsharded inputs (as produced by
reference.setup_inputs()) and returns the FULL [T, B, H] float32 output.

Sharding: pure data-parallel over the batch dim across 8 NeuronCores
(B=4096 -> 512 rows/core); all weights replicated; no collectives.

Per-core kernel (transposed-gates formulation, fp8 recurrence):
  - table[v, :] = S_W*(emb[v] @ W_ih.T) precomputed once (bf16, DRAM);
    b_ih+b_hh is applied later as the per-partition activation bias
  - per step: gatesT = (S_W*W_hh as fp8e4) @ (h^T as fp8e4) on the PE in
    DoubleRow perf mode (2 contraction tiles per instruction, ~2x bf16
    throughput), fp32 accumulate
  - the input-side term arrives pre-transposed via one 512-token
    dma_gather(transpose=True) per step; for gates i,f it is injected
    into PSUM by an identity matmul on the PE (start of the accumulation
    group), for gates g,o added on the vector engine - balancing PE vs
    DVE load
  - gate activations on the scalar engine apply scale=1/S_W and the
    per-partition bias; cell state is bf16 so the whole cell-update chain
    runs in the DVE 2x mode
  - the f32 output is reconstructed through a bf16 DRAM scratch + xbar DMA
    transpose + casting DMA, overlapped one step behind the recurrence
"""

import os
import sys

for _p in ("/opt/trn_rl_repo", os.path.expanduser("~/.axon_site/_ro/trn_rl_repo")):
    if os.path.isdir(_p) and _p not in sys.path:
        sys.path.insert(0, _p)

from contextlib import ExitStack

import numpy as np

import concourse.tile as tile
from concourse import bacc, mybir
from concourse.bass_utils import run_bass_kernel_spmd
from concourse.masks import make_identity

FP32 = mybir.dt.float32
BF16 = mybir.dt.bfloat16
F8 = mybir.dt.float8e4
I32 = mybir.dt.int32
I16 = mybir.dt.int16
AF = mybir.ActivationFunctionType
DR = mybir.MatmulPerfMode.DoubleRow
P = 128
SOS = 1

S_W = 128.0   # fp8 weight scale: W_hh ~ U(-1/32,1/32) is below e4m3 min
              # normal (2^-6); scale into normal range, invert at activation

N_CORES = 8
# Problem shape (hardcoded per contest contract)
B_FULL, T_STEPS, E_DIM, H_DIM, V_DIM = 4096, 20, 256, 1024, 1000


def _build(B, T, E, H, V):
    G = 4 * H
    KH = H // P            # contraction tiles over H (= h-dim chunks)
    KE = E // P
    NG = G // 512
    GC = 4 * KH            # 128-row gate chunks of gatesT
    VP = ((V + P - 1) // P) * P
    VT = VP // P
    B16 = B // 16
    GN = min(B, 256)       # tokens per (transposed) gather
    NHALF = B // GN
    GW = GN // 16          # index slots per gather
    BT = B // P
    assert B % P == 0 and H % 512 == 0 and E % P == 0 and KH % 2 == 0

    nc = bacc.Bacc("TRN2", target_bir_lowering=False, debug=False,
                   num_devices=1, num_swdge_queues=2)
    labels = nc.dram_tensor("labels_batch", [B, T], I32, kind="ExternalInput").ap()
    dh = nc.dram_tensor("decoder_hidden", [B, H], FP32, kind="ExternalInput").ap()
    emb = nc.dram_tensor("emb", [V, E], FP32, kind="ExternalInput").ap()
    W_ih = nc.dram_tensor("W_ih", [G, E], FP32, kind="ExternalInput").ap()
    W_hh = nc.dram_tensor("W_hh", [G, H], FP32, kind="ExternalInput").ap()
    b_ih = nc.dram_tensor("b_ih", [G], FP32, kind="ExternalInput").ap()
    b_hh = nc.dram_tensor("b_hh", [G], FP32, kind="ExternalInput").ap()
    hidden = nc.dram_tensor("hidden", [T, B, H], FP32, kind="ExternalOutput").ap()

    w_hh_bf = nc.dram_tensor("w_hh_bf", [G, H], BF16, kind="Internal").ap()
    w_ih_bf = nc.dram_tensor("w_ih_bf", [G, E], BF16, kind="Internal").ap()
    sos_row = nc.dram_tensor("sos_row", [G], BF16, kind="Internal").ap()
    table = nc.dram_tensor("table", [VP, G], BF16, kind="Internal").ap()
    hiddenT = nc.dram_tensor("hiddenT", [T, H, B], BF16, kind="Internal").ap()

    n_even = [n for n in range(0, NG, 2)]
    n_odd = [n for n in range(1, NG, 2)]

    with tile.TileContext(nc) as tc, ExitStack() as ctx:
        small = ctx.enter_context(tc.tile_pool(name="small", bufs=1))
        wpool = ctx.enter_context(tc.tile_pool(name="w", bufs=1))
        hT_pool = ctx.enter_context(tc.tile_pool(name="hT", bufs=2))
        hT8_pool = ctx.enter_context(tc.tile_pool(name="hT8", bufs=2))

        idx16 = small.tile([P, T, B16], I16, tag="idx16")
        sosT = small.tile([P, GC], FP32, tag="sosT")
        biasT = small.tile([P, GC], FP32, tag="biasT")
        ident_bf = small.tile([P, P], BF16, tag="ident_bf")
        w_hhT8 = wpool.tile([P, KH, G], F8, tag="w_hhT8")

        make_identity(nc, ident_bf)

        hT8_prev = hT8_pool.tile([P, KH, B], F8, tag="hT8", name="hT8_0")

        with tc.tile_pool(name="prolog", bufs=1) as prolog, \
             tc.tile_pool(name="prolog2", bufs=2) as prolog2, \
             tc.tile_pool(name="prolog1", bufs=1) as prolog1, \
             tc.tile_pool(name="prolog4", bufs=4) as prolog4, \
             tc.tile_pool(name="pps", bufs=2, space="PSUM") as pps:
            ident = prolog.tile([P, P], FP32, tag="ident")
            make_identity(nc, ident)

            # ---- loads (sync ring: dh, emb, even W_hh; scalar: W_ih, odd)
            dh_sb = []
            for m in range(BT):
                dh_m = prolog4.tile([P, H], FP32, tag="dh", name=f"dh_{m}")
                nc.sync.dma_start(dh_m, dh[m * P:(m + 1) * P, :])
                dh_sb.append(dh_m)
            emb_sb = prolog.tile([P, VT, E], FP32, tag="emb")
            full = V // P
            if V % P:
                nc.vector.memset(emb_sb[:, VT - 1, :], 0.0)
            nc.sync.dma_start(emb_sb[:, :full, :],
                              emb[:full * P].rearrange("(v p) e -> p v e", p=P))
            if V % P:
                nc.sync.dma_start(emb_sb[:V % P, full, :], emb[full * P:])

            wsi, wsh = [], {}
            for c in range(G // 1024):
                ws = prolog2.tile([P, 8, E], FP32, tag="wsi", name=f"wsi_{c}")
                nc.scalar.dma_start(
                    ws, W_ih[c * 1024:(c + 1) * 1024].rearrange("(o p) e -> p o e", p=P))
                wsi.append(ws)
            for n in n_even:
                ws = wsh[n] = prolog1.tile([P, 4, H], FP32, tag="wshe", name=f"wsh_{n}")
                nc.sync.dma_start(
                    ws, W_hh[n * 512:(n + 1) * 512].rearrange("(o p) h -> p o h", p=P))
            for n in n_odd:
                ws = wsh[n] = prolog1.tile([P, 4, H], FP32, tag="wsho", name=f"wsh_{n}")
                nc.gpsimd.dma_start(
                    ws, W_hh[n * 512:(n + 1) * 512].rearrange("(o p) h -> p o h", p=P))
            brow_i = prolog.tile([P, G // P], FP32, tag="brow_i")
            brow_h = prolog.tile([P, G // P], FP32, tag="brow_h")
            nc.sync.dma_start(brow_i, b_ih.rearrange("(x p) -> p x", p=P))
            nc.sync.dma_start(brow_h, b_hh.rearrange("(x p) -> p x", p=P))
            lab32 = prolog2.tile([P, B16, T], I32, tag="lab32")
            labv = labels.rearrange("(bh p) t -> p bh t", p=16)
            for r in range(8):
                nc.sync.dma_start(lab32[16 * r:16 * (r + 1), :, :], labv)

            # ---- vector casts + SWDGE stores; first-needed first
            wbh = {}

            def cast_whh(n):
                wb = wbh[n] = prolog2.tile([P, 4, H], BF16, tag="wbh", name=f"wbh_{n}")
                nc.vector.tensor_scalar_mul(wb, wsh[n], S_W)

            cast_whh(0)
            for c in range(G // 1024):
                wb = prolog2.tile([P, 8, E], BF16, tag="wbi", name=f"wbi_{c}")
                nc.vector.tensor_copy(wb, wsi[c])
                nc.gpsimd.dma_start(
                    w_ih_bf[c * 1024:(c + 1) * 1024].rearrange("(o p) e -> p o e", p=P), wb)
            for n in n_even[1:] + n_odd:
                cast_whh(n)

            # w_ihT via xbar on the scalar ring (gates table m0 -> sosT)
            w_ihT = prolog.tile([P, KE, G], BF16, tag="w_ihT")
            for k in range(KE):
                for c in range(G // 1024):
                    nc.scalar.dma_start_transpose(
                        w_ihT[:, k, c * 1024:(c + 1) * 1024],
                        w_ih_bf[c * 1024:(c + 1) * 1024, k * P:(k + 1) * P])

            # W_hh transposed on the PE from SBUF (no DRAM round-trip: the
            # prologue is ring-bandwidth-bound); fp8 convert split
            # scalar/vector
            def whh_transpose(ns):
                for n in ns:
                    for k in range(KH):
                        wtp = pps.tile([P, 256], FP32, tag="wtp",
                                       name=f"wtp_{n}_{k}")
                        psb = wtp.bitcast(BF16)
                        for o in range(4):
                            nc.tensor.transpose(
                                psb[:, o * P:(o + 1) * P],
                                wbh[n][:, o, k * P:(k + 1) * P], ident_bf)
                        dst = w_hhT8[:, k, n * 512:(n + 1) * 512]
                        if k % 2 == 0:
                            nc.scalar.activation(dst, psb, AF.Copy)
                        else:
                            nc.vector.tensor_copy(dst, psb)

            whh_transpose(n_even)

            # h0 -> hT8_0 (fp8, unscaled: |h0| ~ N(0,1) fits e4m3) on the PE
            hT80v = hT8_prev.rearrange("p k (m r) -> p k m r", r=P)
            for m in range(BT):
                trp = pps.tile([P, H], FP32, tag="tr", name=f"h0t_{m}")
                for k in range(KH):
                    nc.tensor.transpose(trp[:, k * P:(k + 1) * P],
                                        dh_sb[m][:, k * P:(k + 1) * P], ident)
                nc.vector.tensor_copy(hT80v[:, :, m, :],
                                      trp.rearrange("p (k r) -> p k r", r=P))

            # emb -> embT (scaled by S_W so the table is S_W*(x@W_ih.T))
            embT = prolog.tile([P, KE, VP], BF16, tag="embT")
            for e in range(KE):
                trp = pps.tile([P, VP], FP32, tag="tr", name=f"trp_{e}")
                for vt in range(VT):
                    nc.tensor.transpose(trp[:, vt * P:(vt + 1) * P],
                                        emb_sb[:, vt, e * P:(e + 1) * P], ident)
                nc.vector.tensor_scalar_mul(embT[:, e, :], trp, S_W)

            whh_transpose(n_odd)

            # bias for the gate activations (unscaled: applied post-scale)
            nc.vector.tensor_add(biasT, brow_i, brow_h)

            # table[v] = S_W*(emb[v] @ W_ih.T): matmuls on PE; psum->sbuf
            # copies split scalar/vector; stores on SWDGE. m=0 first (it
            # gates sosT and hence step 0's activations).
            for m in range(VT):
                trow = prolog2.tile([P, G], BF16, tag="trow", name=f"trow_{m}")
                for n in range(NG):
                    ps = pps.tile([P, 512], FP32, tag="g2", name=f"tps_{m}_{n}")
                    for e in range(KE):
                        nc.tensor.matmul(ps, embT[:, e, m * P:(m + 1) * P],
                                         w_ihT[:, e, n * 512:(n + 1) * 512],
                                         start=(e == 0), stop=(e == KE - 1))
                    sl = trow[:, n * 512:(n + 1) * 512]
                    if m % 2 == 0:
                        nc.scalar.activation(sl, ps, AF.Copy)
                    else:
                        nc.vector.tensor_copy(sl, ps)
                if m == SOS // P:
                    nc.sync.dma_start(sos_row[None, :], trow[SOS % P:SOS % P + 1, :])
                    nc.gpsimd.dma_start(sosT, sos_row.rearrange("(gc p) -> p gc", p=P))
                nc.gpsimd.dma_start(table[m * P:(m + 1) * P, :], trow)

            v16 = lab32.bitcast(I16).rearrange(
                "p b (t two) -> p t b two", two=2)[:, :, :, 0]
            nc.vector.tensor_copy(idx16, v16)

        # ---- main-loop pools (prologue SBUF/PSUM freed)
        state = ctx.enter_context(tc.tile_pool(name="state", bufs=1))
        cT = state.tile([P, KH, B], BF16, tag="cT")
        gt_pool = ctx.enter_context(tc.tile_pool(name="gt", bufs=2 * NHALF))
        act_pool = ctx.enter_context(tc.tile_pool(name="act", bufs=16))
        tmp_pool = ctx.enter_context(tc.tile_pool(name="tmp", bufs=2))
        hn_pool = ctx.enter_context(tc.tile_pool(name="hn", bufs=4))
        psum = ctx.enter_context(tc.tile_pool(name="ps", bufs=2, space="PSUM"))

        nc.vector.memset(cT, 0.0)

        def do_gather(t):
            gs = []
            for h in range(NHALF):
                g = gt_pool.tile([P, GC, GN], BF16, tag="gt", name=f"gt_{t}_{h}")
                idxs = idx16[:, t - 1, h * GW:(h + 1) * GW]
                nc.gpsimd.dma_gather(g, table, idxs, num_idxs=GN,
                                     num_idxs_reg=GN, elem_size=G, transpose=True,
                                     queue_num=h % 2)
                gs.append(g)
            return gs

        def emit_output(t):
            for m in range(BT):
                hn = hn_pool.tile([P, H], BF16, tag="hn", name=f"hn_{t}_{m}")
                nc.sync.dma_start_transpose(hn, hiddenT[t][:, m * P:(m + 1) * P])
                nc.gpsimd.dma_start(hidden[t, m * P:(m + 1) * P, :], hn)

        hTd = hiddenT.rearrange("t (k p) b -> t p k b", p=P)
        gts = None
        for t in range(T):
            hT_new = hT_pool.tile([P, KH, B], BF16, tag="hT", name=f"hT_{t + 1}")
            hT8_new = hT8_pool.tile([P, KH, B], F8, tag="hT8", name=f"hT8_{t + 1}")
            for hc in range(KH):
                gif = psum.tile([P, 2, B], FP32, tag="gif", name=f"gif_{t}_{hc}")
                gg = psum.tile([P, B], FP32, tag="gg", name=f"gg_{t}_{hc}")
                go = psum.tile([P, B], FP32, tag="go", name=f"go_{t}_{hc}")
                slots = [gif[:, 0, :], gif[:, 1, :], gg, go]
                for gate in range(4):
                    gc = gate * KH + hc
                    ps = slots[gate]
                    for kp in range(0, KH, 2):
                        nc.tensor.matmul(ps, w_hhT8[:, kp:kp + 2, gc * P:(gc + 1) * P],
                                         hT8_prev[:, kp:kp + 2, :],
                                         start=(kp == 0),
                                         stop=(kp == KH - 2), perf_mode=DR)
                if t == 0:
                    for gate in range(4):
                        gc = gate * KH + hc
                        nc.vector.tensor_scalar_add(slots[gate], slots[gate],
                                                    sosT[:, gc:gc + 1])
                else:
                    # (i,f) pair merged into one strided add per gather half
                    for h in range(NHALF):
                        gv = gts[h]
                        sl = slice(h * GN, (h + 1) * GN)
                        nc.vector.tensor_add(gif[:, :, sl], gif[:, :, sl],
                                             gv[:, hc:2 * KH:KH, :])
                        nc.vector.tensor_add(gg[:, sl], gg[:, sl],
                                             gv[:, 2 * KH + hc, :])
                        nc.vector.tensor_add(go[:, sl], go[:, sl],
                                             gv[:, 3 * KH + hc, :])
                sig = []
                for gate in range(4):
                    gc = gate * KH + hc
                    a = act_pool.tile([P, B], BF16, tag="act", name=f"act_{t}_{gc}")
                    nc.scalar.activation(a, slots[gate],
                                         AF.Tanh if gate == 2 else AF.Sigmoid,
                                         bias=biasT[:, gc:gc + 1], scale=1.0 / S_W)
                    sig.append(a)
                cs = cT[:, hc, :]
                tmp = tmp_pool.tile([P, B], BF16, tag="tmp", name=f"tmp_{t}_{hc}")
                nc.vector.tensor_mul(tmp, sig[0], sig[2])
                nc.vector.tensor_mul(cs, sig[1], cs)
                nc.vector.tensor_add(cs, cs, tmp)
                tca = act_pool.tile([P, B], BF16, tag="act", name=f"tc_{t}_{hc}")
                nc.scalar.activation(tca, cs, AF.Tanh)
                nc.vector.tensor_mul(hT_new[:, hc, :], sig[3], tca)
                nc.scalar.activation(hT8_new[:, hc, :], hT_new[:, hc, :], AF.Copy)
            if t < T - 1:
                nc.sync.dma_start(hTd[t], hT_new)
                if t > 0:
                    emit_output(t - 1)
                gts = do_gather(t + 1)
            else:
                emit_output(t - 1)
                # last step: PE-transpose hT back to natural layout directly
                for m in range(BT):
                    hnat = hn_pool.tile([P, H], FP32, tag="hnat", name=f"hnat_{m}")
                    ps4 = psum.tile([P, 2, B], FP32, tag="gif", name=f"lt_{m}")
                    psb = ps4[:, 0, :].bitcast(BF16)
                    for k in range(KH):
                        nc.tensor.transpose(psb[:, k * P:(k + 1) * P],
                                            hT_new[:, k, m * P:(m + 1) * P], ident_bf)
                    nc.vector.tensor_copy(hnat, psb)
                    nc.sync.dma_start(hidden[t, m * P:(m + 1) * P, :], hnat)
            hT8_prev = hT8_new

    nc.compile()
    return nc


_NC_CACHE = {}


def _get_nc():
    key = (B_FULL, T_STEPS, E_DIM, H_DIM, V_DIM)
    if key not in _NC_CACHE:
        _NC_CACHE[key] = _build(B_FULL // N_CORES, T_STEPS, E_DIM, H_DIM, V_DIM)
    return _NC_CACHE[key]


def kernel(labels_batch, decoder_hidden, emb, W_ih, W_hh, b_ih, b_hh):
    labels_batch = np.ascontiguousarray(np.asarray(labels_batch, dtype=np.int32))
    decoder_hidden = np.ascontiguousarray(np.asarray(decoder_hidden, dtype=np.float32))
    emb = np.ascontiguousarray(np.asarray(emb, dtype=np.float32))
    W_ih = np.ascontiguousarray(np.asarray(W_ih, dtype=np.float32))
    W_hh = np.ascontiguousarray(np.asarray(W_hh, dtype=np.float32))
    b_ih = np.ascontiguousarray(np.asarray(b_ih, dtype=np.float32))
    b_hh = np.ascontiguousarray(np.asarray(b_hh, dtype=np.float32))

    B = B_FULL // N_CORES
    nc = _get_nc()
    in_maps = [{
        "labels_batch": np.ascontiguousarray(labels_batch[c * B:(c + 1) * B]),
        "decoder_hidden": np.ascontiguousarray(decoder_hidden[c * B:(c + 1) * B]),
        "emb": emb,
        "W_ih": W_ih,
        "W_hh": W_hh,
        "b_ih": b_ih,
        "b_hh": b_hh,
    } for c in range(N_CORES)]
    res = run_bass_kernel_spmd(nc, in_maps, core_ids=list(range(N_CORES)))
    return np.concatenate([res.results[c]["hidden"] for c in range(N_CORES)], axis=1)
